# revision 4
# baseline (speedup 1.0000x reference)
"""AttnDecoderRNN single-step kernel for 8 TRN2 NeuronCores.

Tensor-parallel sharding (vocab-TP per the hint):
  - out_W/out_b sharded along vocab (padded 50257 -> 51200, 6400/core),
    log_softmax via a global-stats AllGather (max & exp-sum).
  - comb_W output-sharded (256 H-cols/core); GRU W_ih/W_hh K-sharded on the
    matching 256-wide rnn/h slice, partial gates combined with one AllReduce.
  - Small attention path (attn_W, encoder_outputs) replicated; embedding row
    selected on-device from a 128-row page via a one-hot matmul.

All vectors live partition-major ([128, n] tiles, element c*128+p at [p, c]).
Host pre-tiles every weight into SBUF-ready [128, F] stationary-operand
layouts; big weights are cast to bf16 (f32 PSUM accumulation throughout).
"""

import numpy as np
import ml_dtypes

H = 2048
V = 50257
L = 100
N_CORES = 8
VP = 51200          # padded vocab
VS = VP // N_CORES  # 6400 per core
VB = VS // 128      # 50 vocab blocks per core
KC = H // 128       # 16 k-chunks
GV = 5              # vocab blocks per DMA group
NG = VB // GV       # 10 groups
HC = H // 128       # 16 h chunks
CS = H // N_CORES // 128  # 2 chunks of 128 = per-core 256-slice
NEG = -1.0e30

BF16 = ml_dtypes.bfloat16

_CACHE = {}


def _build():
    import concourse.bacc as bacc
    import concourse.tile as tile
    import concourse.mybir as mybir
    from concourse.masks import make_identity

    F32 = mybir.dt.float32
    BF = mybir.dt.bfloat16
    AX = mybir.AxisListType.X
    AF = mybir.ActivationFunctionType

    nc = bacc.Bacc("TRN2", target_bir_lowering=False, debug=False,
                   num_devices=N_CORES)

    # ---- I/O -----------------------------------------------------------
    d_page = nc.dram_tensor("page", [128, H], F32, kind="ExternalInput")
    d_oneh = nc.dram_tensor("oneh", [128, 1], F32, kind="ExternalInput")
    d_h0 = nc.dram_tensor("h0", [128, HC], F32, kind="ExternalInput")
    d_h0s = nc.dram_tensor("h0s", [128, CS], F32, kind="ExternalInput")
    d_enc = nc.dram_tensor("enc", [L, H], F32, kind="ExternalInput")
    d_attnw = nc.dram_tensor("attnw", [128, 32 * L], F32, kind="ExternalInput")
    d_attnb = nc.dram_tensor("attnb", [L, 1], F32, kind="ExternalInput")
    d_comb = nc.dram_tensor("comb", [128, 32 * CS * 128], BF, kind="ExternalInput")
    d_combb = nc.dram_tensor("combb", [128, CS], F32, kind="ExternalInput")
    d_ih = nc.dram_tensor("ihw", [128, CS * 48 * 128], BF, kind="ExternalInput")
    d_hh = nc.dram_tensor("hhw", [128, CS * 48 * 128], BF, kind="ExternalInput")
    d_gbias = nc.dram_tensor("gbias", [128, 64], F32, kind="ExternalInput")
    d_outw = nc.dram_tensor("outw", [128, VB * KC * 128], BF, kind="ExternalInput")
    d_outb = nc.dram_tensor("outb", [128, VB], F32, kind="ExternalInput")

    d_lp = nc.dram_tensor("lp", [128, VB], F32, kind="ExternalOutput")
    d_h = nc.dram_tensor("h", [128, HC], F32, kind="ExternalOutput")
    d_aw = nc.dram_tensor("aw", [L, 1], F32, kind="ExternalOutput")

    with tile.TileContext(nc) as tc:
        with tc.tile_pool(name="wts", bufs=1) as wts, \
             tc.tile_pool(name="vec", bufs=1) as vec, \
             tc.tile_pool(name="ring", bufs=3) as ring, \
             tc.tile_pool(name="ps_small", bufs=2, space="PSUM") as ps_small, \
             tc.tile_pool(name="ps_gate", bufs=1, space="PSUM") as ps_gate, \
             tc.tile_pool(name="ps_lg", bufs=1, space="PSUM") as ps_lg, \
             tc.tile_pool(name="dram", bufs=1, space="DRAM") as dram:

            # ---- dummy collective first: absorbs collective-entry cost ----
            z1 = vec.tile([1, 1], F32)
            nc.vector.memset(z1[:], 0.0)
            cc_din = dram.tile([1, 1], F32)
            cc_dout = dram.tile([1, 1], F32)
            nc.gpsimd.dma_start(cc_din[:], z1[:])
            nc.gpsimd.collective_compute(
                "AllReduce", mybir.AluOpType.add,
                replica_groups=[list(range(N_CORES))],
                ins=[cc_din.opt()], outs=[cc_dout.opt()])

            # ---- small-path weight DMAs (gpsimd, program order) ----------
            sb_page = wts.tile([128, H], F32)
            nc.gpsimd.dma_start(sb_page[:], d_page[:])
            sb_oneh = wts.tile([128, 1], F32)
            nc.gpsimd.dma_start(sb_oneh[:], d_oneh[:])
            sb_h0 = wts.tile([128, HC], F32)
            nc.gpsimd.dma_start(sb_h0[:], d_h0[:])
            sb_h0s = wts.tile([128, CS], F32)
            nc.gpsimd.dma_start(sb_h0s[:], d_h0s[:])
            sb_attnw = wts.tile([128, 32 * L], F32)
            nc.gpsimd.dma_start(sb_attnw[:], d_attnw[:])
            sb_attnb = wts.tile([L, 1], F32)
            nc.gpsimd.dma_start(sb_attnb[:], d_attnb[:])
            sb_enc = wts.tile([L, H], F32)
            nc.gpsimd.dma_start(sb_enc[:], d_enc[:])
            sb_comb = wts.tile([128, 32 * CS * 128], BF)
            nc.gpsimd.dma_start(sb_comb[:], d_comb[:])
            sb_combb = wts.tile([128, CS], F32)
            nc.gpsimd.dma_start(sb_combb[:], d_combb[:])
            sb_ih = wts.tile([128, CS * 48 * 128], BF)
            nc.gpsimd.dma_start(sb_ih[:], d_ih[:])
            sb_hh = wts.tile([128, CS * 48 * 128], BF)
            nc.gpsimd.dma_start(sb_hh[:], d_hh[:])
            sb_gbias = wts.tile([128, 64], F32)
            nc.gpsimd.dma_start(sb_gbias[:], d_gbias[:])
            sb_outb = wts.tile([128, VB], F32)
            nc.gpsimd.dma_start(sb_outb[:], d_outb[:])

            # constants
            ident = wts.tile([128, 128], F32)
            make_identity(nc, ident[:])
            ones_c = wts.tile([128, 1], F32)
            nc.vector.memset(ones_c[:], 1.0)
            ones_r = wts.tile([1, 128], F32)
            nc.vector.memset(ones_r[:], 1.0)

            # ---- out_W ring DMAs on sync engine (own HWDGE ring) --------
            GRPC = GV * KC * 128  # cols per group
            ring_tiles = []
            for g in range(NG):
                rt = ring.tile([128, GRPC], BF, tag="ring")
                nc.sync.dma_start(rt[:], d_outw[:, g * GRPC:(g + 1) * GRPC])
                ring_tiles.append(rt)

            # ---- embedding select: emb[p_row] via one-hot ---------------
            ps_emb = ps_small.tile([128, HC], F32, tag="sm")
            for c in range(HC):
                nc.tensor.matmul(ps_emb[:, c:c + 1],
                                 sb_page[:, c * 128:(c + 1) * 128],
                                 sb_oneh[:], start=True, stop=True)
            emb_f = vec.tile([128, HC], F32)
            nc.scalar.copy(emb_f[:], ps_emb[:])
            emb_b = vec.tile([128, HC], BF)
            nc.vector.tensor_copy(emb_b[:], ps_emb[:])

            # ---- attention logits [100,1] (W stationary, f32) -----------
            ps_al = ps_small.tile([L, 1], F32, tag="sm")
            for c in range(32):
                rhs = emb_f[:, c:c + 1] if c < HC else sb_h0[:, c - HC:c - HC + 1]
                nc.tensor.matmul(ps_al[:],
                                 sb_attnw[:, c * L:(c + 1) * L],
                                 rhs, start=(c == 0), stop=(c == 31))
            # softmax over 100 partitions (no max-shift; logits are O(1))
            al_e = vec.tile([L, 1], F32)
            nc.scalar.activation(al_e[:], ps_al[:], AF.Exp, bias=sb_attnb[:])
            ps_s = ps_small.tile([1, 1], F32, tag="sm")
            nc.tensor.matmul(ps_s[:], al_e[:], ones_c[:L, :], start=True, stop=True)
            rs = vec.tile([1, 1], F32)
            nc.vector.reciprocal(rs[:], ps_s[:])
            ps_rb = ps_small.tile([L, 1], F32, tag="sm")
            nc.tensor.matmul(ps_rb[:], ones_r[:, :L], rs[:], start=True, stop=True)
            aw_f = vec.tile([L, 1], F32)
            nc.vector.tensor_mul(aw_f[:], al_e[:], ps_rb[:])
            nc.gpsimd.dma_start(d_aw[:], aw_f[:])

            # ---- context = attn_weights @ enc, partition-major ----------
            ps_ctx = ps_small.tile([128, HC], F32, tag="sm")
            for c in range(HC):
                nc.tensor.matmul(ps_ctx[:, c:c + 1],
                                 sb_enc[:, c * 128:(c + 1) * 128],
                                 aw_f[:], start=True, stop=True)
            ctx_b = vec.tile([128, HC], BF)
            nc.vector.tensor_copy(ctx_b[:], ps_ctx[:])

            # ---- rnn_input slice (comb output-sharded) ------------------
            ps_rnn = ps_small.tile([128, CS], F32, tag="sm")
            for m in range(CS):
                for k in range(32):
                    rhs = emb_b[:, k:k + 1] if k < HC else ctx_b[:, k - HC:k - HC + 1]
                    nc.tensor.matmul(ps_rnn[:, m:m + 1],
                                     sb_comb[:, (k * CS + m) * 128:(k * CS + m + 1) * 128],
                                     rhs, start=(k == 0), stop=(k == 31))
            rnn_f = vec.tile([128, CS], F32)
            nc.vector.tensor_add(rnn_f[:], ps_rnn[:], sb_combb[:])
            rnn_b = vec.tile([128, CS], BF)
            nc.vector.tensor_copy(rnn_b[:], rnn_f[:])
            h0s_b = vec.tile([128, CS], BF)
            nc.vector.tensor_copy(h0s_b[:], sb_h0s[:])

            # ---- GRU partial gates (K-sharded) --------------------------
            ps_gi = ps_gate.tile([128, 48], F32, tag="gi")
            ps_gh = ps_gate.tile([128, 48], F32, tag="gh")
            for m in range(48):
                for k in range(CS):
                    nc.tensor.matmul(ps_gi[:, m:m + 1],
                                     sb_ih[:, (k * 48 + m) * 128:(k * 48 + m + 1) * 128],
                                     rnn_b[:, k:k + 1],
                                     start=(k == 0), stop=(k == CS - 1))
            for m in range(48):
                for k in range(CS):
                    nc.tensor.matmul(ps_gh[:, m:m + 1],
                                     sb_hh[:, (k * 48 + m) * 128:(k * 48 + m + 1) * 128],
                                     h0s_b[:, k:k + 1],
                                     start=(k == 0), stop=(k == CS - 1))
            ghs = vec.tile([128, 48], F32)
            nc.scalar.copy(ghs[:], ps_gh[:])
            gpart = vec.tile([128, 64], F32)
            nc.vector.tensor_add(gpart[:, 0:16], ps_gi[:, 0:16], ghs[:, 0:16])
            nc.vector.tensor_add(gpart[:, 16:32], ps_gi[:, 16:32], ghs[:, 16:32])
            nc.vector.tensor_copy(gpart[:, 32:48], ps_gi[:, 32:48])
            nc.vector.tensor_copy(gpart[:, 48:64], ghs[:, 32:48])

            # ---- AllReduce the partial gates ----------------------------
            cc_gin = dram.tile([128, 64], F32)
            cc_gout = dram.tile([128, 64], F32)
            nc.gpsimd.dma_start(cc_gin[:], gpart[:])
            nc.gpsimd.collective_compute(
                "AllReduce", mybir.AluOpType.add,
                replica_groups=[list(range(N_CORES))],
                ins=[cc_gin.opt()], outs=[cc_gout.opt()])
            gfull = vec.tile([128, 64], F32)
            nc.gpsimd.dma_start(gfull[:], cc_gout[:])
            gb = vec.tile([128, 64], F32)
            nc.vector.tensor_add(gb[:], gfull[:], sb_gbias[:])

            # ---- gates + new hidden state -------------------------------
            r_t = vec.tile([128, 16], F32)
            nc.scalar.activation(r_t[:], gb[:, 0:16], AF.Sigmoid)
            z_t = vec.tile([128, 16], F32)
            nc.scalar.activation(z_t[:], gb[:, 16:32], AF.Sigmoid)
            rd = vec.tile([128, 16], F32)
            nc.vector.tensor_mul(rd[:], r_t[:], gb[:, 48:64])
            cn = vec.tile([128, 16], F32)
            nc.vector.tensor_add(cn[:], gb[:, 32:48], rd[:])
            n_t = vec.tile([128, 16], F32)
            nc.scalar.activation(n_t[:], cn[:], AF.Tanh)
            hmn = vec.tile([128, 16], F32)
            nc.vector.tensor_sub(hmn[:], sb_h0[:], n_t[:])
            zh = vec.tile([128, 16], F32)
            nc.vector.tensor_mul(zh[:], z_t[:], hmn[:])
            h_f = vec.tile([128, 16], F32)
            nc.vector.tensor_add(h_f[:], n_t[:], zh[:])
            nc.gpsimd.dma_start(d_h[:], h_f[:])
            h_b = vec.tile([128, 16], BF)
            nc.vector.tensor_copy(h_b[:], h_f[:])

            # ---- big vocab matmul: logits partition-major ---------------
            ps_l = ps_lg.tile([128, VB], F32, tag="lg")
            for g in range(NG):
                rt = ring_tiles[g]
                for vl in range(GV):
                    vb = g * GV + vl
                    for kc in range(KC):
                        off = (vl * KC + kc) * 128
                        nc.tensor.matmul(ps_l[:, vb:vb + 1],
                                         rt[:, off:off + 128],
                                         h_b[:, kc:kc + 1],
                                         start=(kc == 0), stop=(kc == KC - 1))
            ls = vec.tile([128, VB], F32)
            nc.vector.tensor_add(ls[:], ps_l[:], sb_outb[:])

            # ---- local softmax stats ------------------------------------
            cmax = vec.tile([128, 1], F32)
            nc.vector.reduce_max(cmax[:], ls[:], axis=AX)
            ps_t = ps_small.tile([1, 128], F32, tag="sm")
            nc.tensor.transpose(ps_t[:], cmax[:], ident[:])
            m_i = vec.tile([1, 1], F32)
            nc.vector.reduce_max(m_i[:], ps_t[:], axis=AX)
            nm_i = vec.tile([1, 1], F32)
            nc.scalar.mul(nm_i[:], m_i[:], -1.0)
            ps_nm = ps_small.tile([128, 1], F32, tag="sm")
            nc.tensor.matmul(ps_nm[:], ones_r[:], nm_i[:], start=True, stop=True)
            nmb = vec.tile([128, 1], F32)
            nc.vector.tensor_copy(nmb[:], ps_nm[:])
            e_t = vec.tile([128, VB], F32)
            sacc = vec.tile([128, 1], F32)
            nc.scalar.activation(e_t[:], ls[:], AF.Exp, bias=nmb[:],
                                 accum_out=sacc[:])
            ps_s2 = ps_small.tile([1, 1], F32, tag="sm")
            nc.tensor.matmul(ps_s2[:], sacc[:], ones_c[:], start=True, stop=True)
            stat = vec.tile([1, 2], F32)
            nc.vector.tensor_copy(stat[:, 0:1], m_i[:])
            nc.vector.tensor_copy(stat[:, 1:2], ps_s2[:])

            # ---- AllGather stats, combine globally ----------------------
            cc_sin = dram.tile([1, 2], F32)
            cc_sout = dram.tile([N_CORES, 2], F32)
            nc.gpsimd.dma_start(cc_sin[:], stat[:])
            nc.gpsimd.collective_compute(
                "AllGather", mybir.AluOpType.bypass,
                replica_groups=[list(range(N_CORES))],
                ins=[cc_sin.opt()], outs=[cc_sout.opt()])
            st = vec.tile([1, 2 * N_CORES], F32)
            nc.gpsimd.dma_start(st[:], cc_sout[:])
            stv = st[:].rearrange("p (j k) -> p j k", k=2)
            gmax = vec.tile([1, 1], F32)
            nc.vector.reduce_max(gmax[:], stv[:, :, 0], axis=AX)
            ngmax = vec.tile([1, 1], F32)
            nc.scalar.mul(ngmax[:], gmax[:], -1.0)
            tj = vec.tile([1, N_CORES], F32)
            nc.scalar.activation(tj[:], stv[:, :, 0], AF.Exp, bias=ngmax[:])
            wj = vec.tile([1, N_CORES], F32)
            nc.vector.tensor_mul(wj[:], tj[:], stv[:, :, 1])
            gsum = vec.tile([1, 1], F32)
            nc.vector.reduce_sum(gsum[:], wj[:], axis=AX)
            lgs = vec.tile([1, 1], F32)
            nc.scalar.activation(lgs[:], gsum[:], AF.Ln)
            logz = vec.tile([1, 1], F32)
            nc.vector.tensor_add(logz[:], lgs[:], gmax[:])
            ps_z = ps_small.tile([128, 1], F32, tag="sm")
            nc.tensor.matmul(ps_z[:], ones_r[:], logz[:], start=True, stop=True)
            zb = vec.tile([128, 1], F32)
            nc.vector.tensor_copy(zb[:], ps_z[:])
            lp_t = vec.tile([128, VB], F32)
            nc.vector.tensor_scalar_sub(lp_t[:], ls[:], zb[:])
            nc.gpsimd.dma_start(d_lp[:], lp_t[:])

    nc.compile()
    return nc


def _pm(x):
    """[n*128] -> [128, n] partition-major."""
    n = x.shape[-1] // 128
    return np.ascontiguousarray(x.reshape(n, 128).T)


def _prep(input_tok, hidden, encoder_outputs, emb, attn_W, attn_b,
          comb_W, comb_b, W_ih, b_ih, W_hh, b_hh, out_W, out_b):
    tok = int(np.asarray(input_tok).reshape(-1)[0])
    blk = min((tok // 128) * 128, V - 128)
    page = np.ascontiguousarray(emb[blk:blk + 128]).astype(np.float32)
    oneh = np.zeros((128, 1), np.float32)
    oneh[tok - blk, 0] = 1.0

    h0 = np.asarray(hidden, np.float32).reshape(H)
    h0_pm = _pm(h0)

    attnw_t = np.ascontiguousarray(
        np.asarray(attn_W, np.float32).reshape(32, 128, L)
        .transpose(1, 0, 2).reshape(128, 32 * L))
    attnb_t = np.asarray(attn_b, np.float32).reshape(L, 1)
    enc = np.ascontiguousarray(np.asarray(encoder_outputs, np.float32))

    # padded out_W/out_b
    oW = np.zeros((H, VP), np.float32)
    oW[:, :V] = out_W
    ob = np.full((VP,), NEG, np.float32)
    ob[:V] = out_b

    gb = np.zeros((64 * 128,), np.float32)
    gb[0:2048] = b_ih[0:H] + b_hh[0:H]
    gb[2048:4096] = b_ih[H:2 * H] + b_hh[H:2 * H]
    gb[4096:6144] = b_ih[2 * H:3 * H]
    gb[6144:8192] = b_hh[2 * H:3 * H]
    gbias = _pm(gb)

    in_maps = []
    for i in range(N_CORES):
        ci = slice(i * 256, (i + 1) * 256)
        comb_i = np.ascontiguousarray(
            np.asarray(comb_W[:, ci], np.float32).reshape(32, 128, CS, 128)
            .transpose(1, 0, 2, 3).reshape(128, 32 * CS * 128)).astype(BF16)
        combb_i = _pm(np.asarray(comb_b[ci], np.float32))
        ih_i = np.ascontiguousarray(
            np.asarray(W_ih[:, ci], np.float32).T.reshape(CS, 128, 48, 128)
            .transpose(1, 0, 2, 3).reshape(128, CS * 48 * 128)).astype(BF16)
        hh_i = np.ascontiguousarray(
            np.asarray(W_hh[:, ci], np.float32).T.reshape(CS, 128, 48, 128)
            .transpose(1, 0, 2, 3).reshape(128, CS * 48 * 128)).astype(BF16)
        vi = slice(i * VS, (i + 1) * VS)
        ow_i = np.ascontiguousarray(
            oW[:, vi].reshape(KC, 128, VB, 128)
            .transpose(1, 2, 0, 3).reshape(128, VB * KC * 128)).astype(BF16)
        ob_i = np.ascontiguousarray(ob[vi].reshape(VB, 128).T)
        h0s_i = np.ascontiguousarray(h0_pm[:, 2 * i:2 * i + 2])
        in_maps.append({
            "page": page, "oneh": oneh, "h0": h0_pm, "h0s": h0s_i,
            "enc": enc, "attnw": attnw_t, "attnb": attnb_t,
            "comb": comb_i, "combb": combb_i, "ihw": ih_i, "hhw": hh_i,
            "gbias": gbias, "outw": ow_i, "outb": ob_i,
        })
    return in_maps


def run_spmd(in_maps, trace=False):
    from concourse.bass_utils import run_bass_kernel_spmd
    if "nc" not in _CACHE:
        _CACHE["nc"] = _build()
    return run_bass_kernel_spmd(_CACHE["nc"], in_maps,
                                core_ids=list(range(N_CORES)), trace=trace)


def kernel(input_tok, hidden, encoder_outputs, emb, attn_W, attn_b,
           comb_W, comb_b, W_ih, b_ih, W_hh, b_hh, out_W, out_b):
    in_maps = _prep(input_tok, hidden, encoder_outputs, emb, attn_W, attn_b,
                    comb_W, comb_b, W_ih, b_ih, W_hh, b_hh, out_W, out_b)
    res = run_spmd(in_maps)
    outs = res.results
    lp = np.concatenate([outs[i]["lp"].T.reshape(-1) for i in range(N_CORES)])
    logp = lp[:V].reshape(1, V).astype(np.float32)
    h_new = outs[0]["h"].T.reshape(1, 1, H).astype(np.float32)
    attnw = outs[0]["aw"].reshape(1, L).astype(np.float32)
    return logp, h_new, attnw


# revision 7
# speedup vs baseline: 1.0118x; 1.0118x over previous
"""AttnDecoderRNN single-step kernel for 8 TRN2 NeuronCores.

Tensor-parallel sharding (vocab-TP per the hint):
  - out_W/out_b sharded along vocab (padded 50257 -> 51200, 6400/core),
    log_softmax via a global-stats AllGather (max & exp-sum).
  - comb_W output-sharded (256 H-cols/core); GRU W_ih/W_hh K-sharded on the
    matching 256-wide rnn/h slice, partial gates combined with one AllReduce.
  - Small attention path (attn_W, encoder_outputs) replicated; embedding row
    selected on-device from a 128-row page via a one-hot matmul.

Small-path vectors live partition-major ([128, n] tiles). The vocab matmul
keeps h stationary and streams out_W as the moving operand (N=512), with
out_b folded in as an extra K=1 accumulation row; logits land on the free
dim of partition 0 where the local softmax stats are computed. Weights are
host-pre-tiled into SBUF-ready layouts, big ones cast to bf16 (f32 PSUM).
"""

import numpy as np
import ml_dtypes

H = 2048
V = 50257
L = 100
N_CORES = 8
VP = 51200          # padded vocab
VS = VP // N_CORES  # 6400 per core
KC = H // 128       # 16 k-chunks
HC = H // 128       # 16 h chunks
CS = H // N_CORES // 128  # 2 chunks of 128 = per-core 256-slice
NEG = -1.0e30

# vocab j-tiles: 12x512 + 1x256, split into two PSUM passes (7 + 6 tiles)
TILES = [(j, 512) for j in range(12)] + [(12, 256)]
PASS0 = TILES[:7]           # cols [0, 3584)
PASS1 = TILES[7:]           # cols [3584, 6400)
C0 = sum(w for _, w in PASS0)   # 3584
C1 = sum(w for _, w in PASS1)   # 2816

BF16 = ml_dtypes.bfloat16

_CACHE = {}


def _build():
    import concourse.bacc as bacc
    import concourse.tile as tile
    import concourse.mybir as mybir

    F32 = mybir.dt.float32
    BF = mybir.dt.bfloat16
    AX = mybir.AxisListType.X
    AF = mybir.ActivationFunctionType

    nc = bacc.Bacc("TRN2", target_bir_lowering=False, debug=False,
                   num_devices=N_CORES)

    # ---- I/O -----------------------------------------------------------
    d_page = nc.dram_tensor("page", [128, H], BF, kind="ExternalInput")
    d_oneh = nc.dram_tensor("oneh", [128, 1], BF, kind="ExternalInput")
    d_h0 = nc.dram_tensor("h0", [128, HC], F32, kind="ExternalInput")
    d_h0s = nc.dram_tensor("h0s", [128, CS], BF, kind="ExternalInput")
    d_enc = nc.dram_tensor("enc", [L, H], BF, kind="ExternalInput")
    d_attnw = nc.dram_tensor("attnw", [128, 32 * L], BF, kind="ExternalInput")
    d_attnb = nc.dram_tensor("attnb", [L, 1], F32, kind="ExternalInput")
    d_comb = nc.dram_tensor("comb", [128, 32 * CS * 128], BF, kind="ExternalInput")
    d_combb = nc.dram_tensor("combb", [128, CS], F32, kind="ExternalInput")
    d_ih = nc.dram_tensor("ihw", [128, CS * 48 * 128], BF, kind="ExternalInput")
    d_hh = nc.dram_tensor("hhw", [128, CS * 48 * 128], BF, kind="ExternalInput")
    d_gbias = nc.dram_tensor("gbias", [128, 64], F32, kind="ExternalInput")
    d_ow0 = nc.dram_tensor("outw0", [8, 2, 128, C0], BF, kind="ExternalInput")
    d_ow1 = nc.dram_tensor("outw1", [8, 2, 128, C1], BF, kind="ExternalInput")
    d_outb = nc.dram_tensor("outb", [1, VS], BF, kind="ExternalInput")

    d_lp = nc.dram_tensor("lp", [1, VS], F32, kind="ExternalOutput")
    d_h = nc.dram_tensor("h", [128, HC], F32, kind="ExternalOutput")
    d_aw = nc.dram_tensor("aw", [L, 1], F32, kind="ExternalOutput")

    with tile.TileContext(nc) as tc:
        with tc.tile_pool(name="wts", bufs=1) as wts, \
             tc.tile_pool(name="vec", bufs=1) as vec, \
             tc.tile_pool(name="ring", bufs=4) as ring, \
             tc.tile_pool(name="ps_sm", bufs=1, space="PSUM") as ps_sm, \
             tc.tile_pool(name="ps_j", bufs=1, space="PSUM") as ps_j, \
             tc.tile_pool(name="dram", bufs=1, space="DRAM") as dram:

            # ---- dummy collective first: absorbs start skew + cold mesh --
            cc_din = dram.tile([1, 1], F32)
            cc_dout = dram.tile([1, 1], F32)
            nc.gpsimd.collective_compute(
                "AllReduce", mybir.AluOpType.add,
                replica_groups=[list(range(N_CORES))],
                ins=[cc_din.opt()], outs=[cc_dout.opt()])

            # ---- small-path weight DMAs (gpsimd SWDGE, program order) ----
            sb_page = wts.tile([128, H], BF)
            nc.gpsimd.dma_start(sb_page[:], d_page[:])
            sb_oneh = wts.tile([128, 1], BF)
            nc.gpsimd.dma_start(sb_oneh[:], d_oneh[:])
            sb_h0 = wts.tile([128, HC], F32)
            nc.gpsimd.dma_start(sb_h0[:], d_h0[:])
            sb_h0s = wts.tile([128, CS], BF)
            nc.gpsimd.dma_start(sb_h0s[:], d_h0s[:])
            sb_attnw = wts.tile([128, 32 * L], BF)
            nc.gpsimd.dma_start(sb_attnw[:], d_attnw[:])
            sb_attnb = wts.tile([L, 1], F32)
            nc.gpsimd.dma_start(sb_attnb[:], d_attnb[:])
            sb_enc = wts.tile([L, H], BF)
            nc.gpsimd.dma_start(sb_enc[:], d_enc[:])
            sb_comb = wts.tile([128, 32 * CS * 128], BF)
            nc.gpsimd.dma_start(sb_comb[:], d_comb[:])
            sb_combb = wts.tile([128, CS], F32)
            nc.gpsimd.dma_start(sb_combb[:], d_combb[:])
            sb_ih = wts.tile([128, CS * 48 * 128], BF)
            nc.gpsimd.dma_start(sb_ih[:], d_ih[:])
            sb_hh = wts.tile([128, CS * 48 * 128], BF)
            nc.gpsimd.dma_start(sb_hh[:], d_hh[:])
            sb_gbias = wts.tile([128, 64], F32)
            nc.gpsimd.dma_start(sb_gbias[:], d_gbias[:])
            sb_outb = wts.tile([1, VS], BF)
            nc.gpsimd.dma_start(sb_outb[:], d_outb[:])

            # constants
            ones_c = wts.tile([128, 1], F32)
            nc.vector.memset(ones_c[:], 1.0)
            ones_r = wts.tile([1, 128], F32)
            nc.vector.memset(ones_r[:], 1.0)
            one1b = wts.tile([1, 1], BF)
            nc.vector.memset(one1b[:], 1.0)

            # ---- out_W ring DMAs on sync engine (own HWDGE ring) --------
            ring_tiles = {}
            for p, (d_ow, Cp) in enumerate([(d_ow0, C0), (d_ow1, C1)]):
                for g in range(8):
                    rt = ring.tile([128, 2 * C0], BF, tag="ring")
                    src = d_ow[g].rearrange("two prt c -> prt two c")
                    dst = rt[:, :2 * Cp].rearrange("prt (two c) -> prt two c",
                                                   two=2)
                    nc.sync.dma_start(dst, src)
                    ring_tiles[(p, g)] = rt

            # ---- embedding select: emb row via one-hot ------------------
            ps_emb = ps_sm.tile([128, HC], F32, tag="sm")
            for c in range(HC):
                nc.tensor.matmul(ps_emb[:, c:c + 1],
                                 sb_page[:, c * 128:(c + 1) * 128],
                                 sb_oneh[:], start=True, stop=True)
            emb_b = vec.tile([128, HC], BF)
            nc.vector.tensor_copy(emb_b[:], ps_emb[:])
            h0_b = vec.tile([128, HC], BF)
            nc.vector.tensor_copy(h0_b[:], sb_h0[:])

            # ---- attention logits [100,1] (W stationary) ----------------
            ps_al = ps_sm.tile([L, 1], F32, tag="sm")
            for c in range(32):
                rhs = emb_b[:, c:c + 1] if c < HC else h0_b[:, c - HC:c - HC + 1]
                nc.tensor.matmul(ps_al[:],
                                 sb_attnw[:, c * L:(c + 1) * L],
                                 rhs, start=(c == 0), stop=(c == 31))
            # softmax over 100 partitions (no max-shift; logits are O(1))
            al_e = vec.tile([L, 1], F32)
            nc.scalar.activation(al_e[:], ps_al[:], AF.Exp, bias=sb_attnb[:])
            ps_s = ps_sm.tile([1, 1], F32, tag="sm")
            nc.tensor.matmul(ps_s[:], al_e[:], ones_c[:L, :], start=True, stop=True)
            rs = vec.tile([1, 1], F32)
            nc.vector.reciprocal(rs[:], ps_s[:])
            ps_rb = ps_sm.tile([L, 1], F32, tag="sm")
            nc.tensor.matmul(ps_rb[:], ones_r[:, :L], rs[:], start=True, stop=True)
            aw_f = vec.tile([L, 1], F32)
            nc.vector.tensor_mul(aw_f[:], al_e[:], ps_rb[:])
            nc.gpsimd.dma_start(d_aw[:], aw_f[:])
            aw_b = vec.tile([L, 1], BF)
            nc.vector.tensor_copy(aw_b[:], aw_f[:])

            # ---- context = attn_weights @ enc, partition-major ----------
            ps_ctx = ps_sm.tile([128, HC], F32, tag="sm")
            for c in range(HC):
                nc.tensor.matmul(ps_ctx[:, c:c + 1],
                                 sb_enc[:, c * 128:(c + 1) * 128],
                                 aw_b[:], start=True, stop=True)
            ctx_b = vec.tile([128, HC], BF)
            nc.vector.tensor_copy(ctx_b[:], ps_ctx[:])

            # ---- rnn_input slice (comb output-sharded) ------------------
            ps_rnn = ps_sm.tile([128, CS], F32, tag="sm")
            for m in range(CS):
                for k in range(32):
                    rhs = emb_b[:, k:k + 1] if k < HC else ctx_b[:, k - HC:k - HC + 1]
                    nc.tensor.matmul(ps_rnn[:, m:m + 1],
                                     sb_comb[:, (k * CS + m) * 128:(k * CS + m + 1) * 128],
                                     rhs, start=(k == 0), stop=(k == 31))
            rnn_f = vec.tile([128, CS], F32)
            nc.vector.tensor_add(rnn_f[:], ps_rnn[:], sb_combb[:])
            rnn_b = vec.tile([128, CS], BF)
            nc.vector.tensor_copy(rnn_b[:], rnn_f[:])

            # ---- GRU partial gates (K-sharded) --------------------------
            ps_gi = ps_j.tile([128, 48], F32, tag="j0")
            ps_gh = ps_j.tile([128, 48], F32, tag="j1")
            for m in range(48):
                for k in range(CS):
                    nc.tensor.matmul(ps_gi[:, m:m + 1],
                                     sb_ih[:, (k * 48 + m) * 128:(k * 48 + m + 1) * 128],
                                     rnn_b[:, k:k + 1],
                                     start=(k == 0), stop=(k == CS - 1))
            for m in range(48):
                for k in range(CS):
                    nc.tensor.matmul(ps_gh[:, m:m + 1],
                                     sb_hh[:, (k * 48 + m) * 128:(k * 48 + m + 1) * 128],
                                     sb_h0s[:, k:k + 1],
                                     start=(k == 0), stop=(k == CS - 1))
            ghs = vec.tile([128, 48], F32)
            nc.scalar.copy(ghs[:], ps_gh[:])
            gpart = vec.tile([128, 64], F32)
            nc.vector.tensor_add(gpart[:, 0:16], ps_gi[:, 0:16], ghs[:, 0:16])
            nc.vector.tensor_add(gpart[:, 16:32], ps_gi[:, 16:32], ghs[:, 16:32])
            nc.vector.tensor_copy(gpart[:, 32:48], ps_gi[:, 32:48])
            nc.vector.tensor_copy(gpart[:, 48:64], ghs[:, 32:48])

            # ---- AllReduce the partial gates (bounces on scalar HWDGE) --
            cc_gin = dram.tile([128, 64], F32)
            cc_gout = dram.tile([128, 64], F32)
            nc.scalar.dma_start(cc_gin[:], gpart[:])
            nc.gpsimd.collective_compute(
                "AllReduce", mybir.AluOpType.add,
                replica_groups=[list(range(N_CORES))],
                ins=[cc_gin.opt()], outs=[cc_gout.opt()])
            gfull = vec.tile([128, 64], F32)
            nc.scalar.dma_start(gfull[:], cc_gout[:])
            gb = vec.tile([128, 64], F32)
            nc.vector.tensor_add(gb[:], gfull[:], sb_gbias[:])

            # ---- gates + new hidden state -------------------------------
            r_t = vec.tile([128, 16], F32)
            nc.scalar.activation(r_t[:], gb[:, 0:16], AF.Sigmoid)
            z_t = vec.tile([128, 16], F32)
            nc.scalar.activation(z_t[:], gb[:, 16:32], AF.Sigmoid)
            rd = vec.tile([128, 16], F32)
            nc.vector.tensor_mul(rd[:], r_t[:], gb[:, 48:64])
            cn = vec.tile([128, 16], F32)
            nc.vector.tensor_add(cn[:], gb[:, 32:48], rd[:])
            n_t = vec.tile([128, 16], F32)
            nc.scalar.activation(n_t[:], cn[:], AF.Tanh)
            hmn = vec.tile([128, 16], F32)
            nc.vector.tensor_sub(hmn[:], sb_h0[:], n_t[:])
            zh = vec.tile([128, 16], F32)
            nc.vector.tensor_mul(zh[:], z_t[:], hmn[:])
            h_f = vec.tile([128, 16], F32)
            nc.vector.tensor_add(h_f[:], n_t[:], zh[:])
            nc.gpsimd.dma_start(d_h[:], h_f[:])
            h_b = vec.tile([128, 16], BF)
            nc.vector.tensor_copy(h_b[:], h_f[:])

            # ---- vocab matmul: h stationary, W moving (N<=512) ----------
            ls = vec.tile([1, VS], F32)
            mx = vec.tile([1, 13], F32)
            for p, (tiles_p, Cp) in enumerate([(PASS0, C0), (PASS1, C1)]):
                base = tiles_p[0][0] * 512
                psts = [ps_j.tile([1, w], F32, tag=f"j{i}", name=f"ps_p{p}j{i}")
                        for i, (j, w) in enumerate(tiles_p)]
                for g in range(8):
                    rt = ring_tiles[(p, g)]
                    for kk in range(2):
                        k = 2 * g + kk
                        for i, (j, w) in enumerate(tiles_p):
                            off = kk * Cp + (j * 512 - base)
                            nc.tensor.matmul(psts[i][:],
                                             h_b[:, k:k + 1],
                                             rt[:, off:off + w],
                                             start=(k == 0), stop=False)
                for i, (j, w) in enumerate(tiles_p):
                    # fold out_b in as a K=1 accumulation row
                    nc.tensor.matmul(psts[i][:], one1b[:],
                                     sb_outb[:, j * 512:j * 512 + w],
                                     start=False, stop=True)
                    nc.scalar.copy(ls[:, j * 512:j * 512 + w], psts[i][:])
                    nc.vector.reduce_max(mx[:, j:j + 1], psts[i][:], axis=AX)

            # ---- local softmax stats on [1, VS] -------------------------
            m_i = vec.tile([1, 1], F32)
            nc.vector.reduce_max(m_i[:], mx[:], axis=AX)
            nm_i = vec.tile([1, 1], F32)
            nc.scalar.mul(nm_i[:], m_i[:], -1.0)
            esc = vec.tile([1, 512], F32)
            sacc = vec.tile([1, 13], F32)
            for j, w in TILES:
                nc.scalar.activation(esc[:, :w], ls[:, j * 512:j * 512 + w],
                                     AF.Exp, bias=nm_i[:],
                                     accum_out=sacc[:, j:j + 1])
            s_i = vec.tile([1, 1], F32)
            nc.vector.reduce_sum(s_i[:], sacc[:], axis=AX)
            stat = vec.tile([1, 2], F32)
            nc.vector.tensor_copy(stat[:, 0:1], m_i[:])
            nc.vector.tensor_copy(stat[:, 1:2], s_i[:])

            # ---- AllGather stats, combine globally ----------------------
            cc_sin = dram.tile([1, 2], F32)
            cc_sout = dram.tile([N_CORES, 2], F32)
            nc.scalar.dma_start(cc_sin[:], stat[:])
            nc.gpsimd.collective_compute(
                "AllGather", mybir.AluOpType.bypass,
                replica_groups=[list(range(N_CORES))],
                ins=[cc_sin.opt()], outs=[cc_sout.opt()])
            st = vec.tile([1, 2 * N_CORES], F32)
            nc.scalar.dma_start(st[:], cc_sout[:])
            stv = st[:].rearrange("p (j k) -> p j k", k=2)
            gmax = vec.tile([1, 1], F32)
            nc.vector.reduce_max(gmax[:], stv[:, :, 0], axis=AX)
            ngmax = vec.tile([1, 1], F32)
            nc.scalar.mul(ngmax[:], gmax[:], -1.0)
            tj = vec.tile([1, N_CORES], F32)
            nc.scalar.activation(tj[:], stv[:, :, 0], AF.Exp, bias=ngmax[:])
            wj = vec.tile([1, N_CORES], F32)
            nc.vector.tensor_mul(wj[:], tj[:], stv[:, :, 1])
            gsum = vec.tile([1, 1], F32)
            nc.vector.reduce_sum(gsum[:], wj[:], axis=AX)
            lgs = vec.tile([1, 1], F32)
            nc.scalar.activation(lgs[:], gsum[:], AF.Ln)
            logz = vec.tile([1, 1], F32)
            nc.vector.tensor_add(logz[:], lgs[:], gmax[:])
            nlogz = vec.tile([1, 1], F32)
            nc.scalar.mul(nlogz[:], logz[:], -1.0)
            # logp = ls - logZ, split across DVE and ACT halves, in place
            HLF = 3072
            nc.vector.tensor_scalar_sub(ls[:, :HLF], ls[:, :HLF], logz[:])
            nc.scalar.activation(ls[:, HLF:], ls[:, HLF:], AF.Identity,
                                 bias=nlogz[:])
            nc.gpsimd.dma_start(d_lp[:], ls[:])

    nc.compile()
    return nc


def _pm(x):
    """[n*128] -> [128, n] partition-major."""
    n = x.shape[-1] // 128
    return np.ascontiguousarray(x.reshape(n, 128).T)


def _prep(input_tok, hidden, encoder_outputs, emb, attn_W, attn_b,
          comb_W, comb_b, W_ih, b_ih, W_hh, b_hh, out_W, out_b):
    tok = int(np.asarray(input_tok).reshape(-1)[0])
    blk = min((tok // 128) * 128, V - 128)
    page = np.ascontiguousarray(emb[blk:blk + 128]).astype(BF16)
    oneh = np.zeros((128, 1), BF16)
    oneh[tok - blk, 0] = 1.0

    h0 = np.asarray(hidden, np.float32).reshape(H)
    h0_pm = _pm(h0)

    attnw_t = np.ascontiguousarray(
        np.asarray(attn_W, np.float32).reshape(32, 128, L)
        .transpose(1, 0, 2).reshape(128, 32 * L)).astype(BF16)
    attnb_t = np.asarray(attn_b, np.float32).reshape(L, 1)
    enc = np.ascontiguousarray(np.asarray(encoder_outputs, np.float32)).astype(BF16)

    # padded out_W/out_b
    oW = np.zeros((H, VP), np.float32)
    oW[:, :V] = out_W
    ob = np.full((VP,), NEG, np.float32)
    ob[:V] = out_b

    gb = np.zeros((64 * 128,), np.float32)
    gb[0:2048] = b_ih[0:H] + b_hh[0:H]
    gb[2048:4096] = b_ih[H:2 * H] + b_hh[H:2 * H]
    gb[4096:6144] = b_ih[2 * H:3 * H]
    gb[6144:8192] = b_hh[2 * H:3 * H]
    gbias = _pm(gb)

    in_maps = []
    for i in range(N_CORES):
        ci = slice(i * 256, (i + 1) * 256)
        comb_i = np.ascontiguousarray(
            np.asarray(comb_W[:, ci], np.float32).reshape(32, 128, CS, 128)
            .transpose(1, 0, 2, 3).reshape(128, 32 * CS * 128)).astype(BF16)
        combb_i = _pm(np.asarray(comb_b[ci], np.float32))
        ih_i = np.ascontiguousarray(
            np.asarray(W_ih[:, ci], np.float32).T.reshape(CS, 128, 48, 128)
            .transpose(1, 0, 2, 3).reshape(128, CS * 48 * 128)).astype(BF16)
        hh_i = np.ascontiguousarray(
            np.asarray(W_hh[:, ci], np.float32).T.reshape(CS, 128, 48, 128)
            .transpose(1, 0, 2, 3).reshape(128, CS * 48 * 128)).astype(BF16)
        oWi = oW[:, i * VS:(i + 1) * VS]
        ow0 = np.ascontiguousarray(
            oWi[:, :C0].reshape(8, 2, 128, C0)).astype(BF16)
        ow1 = np.ascontiguousarray(
            oWi[:, C0:].reshape(8, 2, 128, C1)).astype(BF16)
        ob_i = np.ascontiguousarray(ob[i * VS:(i + 1) * VS].reshape(1, VS)).astype(BF16)
        h0s_i = np.ascontiguousarray(h0_pm[:, 2 * i:2 * i + 2]).astype(BF16)
        in_maps.append({
            "page": page, "oneh": oneh, "h0": h0_pm, "h0s": h0s_i,
            "enc": enc, "attnw": attnw_t, "attnb": attnb_t,
            "comb": comb_i, "combb": combb_i, "ihw": ih_i, "hhw": hh_i,
            "gbias": gbias, "outw0": ow0, "outw1": ow1, "outb": ob_i,
        })
    return in_maps


def run_spmd(in_maps, trace=False):
    from concourse.bass_utils import run_bass_kernel_spmd
    if "nc" not in _CACHE:
        _CACHE["nc"] = _build()
    return run_bass_kernel_spmd(_CACHE["nc"], in_maps,
                                core_ids=list(range(N_CORES)), trace=trace)


def kernel(input_tok, hidden, encoder_outputs, emb, attn_W, attn_b,
           comb_W, comb_b, W_ih, b_ih, W_hh, b_hh, out_W, out_b):
    in_maps = _prep(input_tok, hidden, encoder_outputs, emb, attn_W, attn_b,
                    comb_W, comb_b, W_ih, b_ih, W_hh, b_hh, out_W, out_b)
    res = run_spmd(in_maps)
    outs = res.results
    lp = np.concatenate([outs[i]["lp"].reshape(-1) for i in range(N_CORES)])
    logp = lp[:V].reshape(1, V).astype(np.float32)
    h_new = outs[0]["h"].T.reshape(1, 1, H).astype(np.float32)
    attnw = outs[0]["aw"].reshape(1, L).astype(np.float32)
    return logp, h_new, attnw


# revision 8
# speedup vs baseline: 1.0430x; 1.0308x over previous
"""AttnDecoderRNN single-step kernel for 8 TRN2 NeuronCores.

Tensor-parallel sharding (vocab-TP per the hint):
  - out_W/out_b sharded along vocab (padded 50257 -> 51200, 6400/core),
    log_softmax via a global-stats AllGather (max & exp-sum).
  - comb_W output-sharded (256 H-cols/core); GRU W_ih/W_hh K-sharded on the
    matching 256-wide rnn/h slice, partial gates combined with one AllReduce.
  - Small attention path (attn_W, encoder_outputs) replicated; embedding row
    selected on-device from a 128-row page via a one-hot matmul.

Small-path vectors live partition-major ([128, n] tiles). The vocab matmul
keeps h stationary and streams out_W as the moving operand (N=512), with
out_b folded in as an extra K=1 accumulation row; logits land on the free
dim of partition 0 where the local softmax stats are computed. Weights are
host-pre-tiled into SBUF-ready layouts, big ones cast to bf16 (f32 PSUM).
"""

import numpy as np
import ml_dtypes

H = 2048
V = 50257
L = 100
N_CORES = 8
VP = 51200          # padded vocab
VS = VP // N_CORES  # 6400 per core
KC = H // 128       # 16 k-chunks
HC = H // 128       # 16 h chunks
CS = H // N_CORES // 128  # 2 chunks of 128 = per-core 256-slice
NEG = -1.0e30

# vocab j-tiles: 12x512 + 1x256, split into two PSUM passes (7 + 6 tiles)
TILES = [(j, 512) for j in range(12)] + [(12, 256)]
PASS0 = TILES[:7]           # cols [0, 3584)
PASS1 = TILES[7:]           # cols [3584, 6400)
C0 = sum(w for _, w in PASS0)   # 3584
C1 = sum(w for _, w in PASS1)   # 2816

BF16 = ml_dtypes.bfloat16

_CACHE = {}


def _build():
    import concourse.bacc as bacc
    import concourse.tile as tile
    import concourse.mybir as mybir

    F32 = mybir.dt.float32
    BF = mybir.dt.bfloat16
    AX = mybir.AxisListType.X
    AF = mybir.ActivationFunctionType

    nc = bacc.Bacc("TRN2", target_bir_lowering=False, debug=False,
                   num_devices=N_CORES)

    # ---- I/O -----------------------------------------------------------
    d_page = nc.dram_tensor("page", [128, H], BF, kind="ExternalInput")
    d_oneh = nc.dram_tensor("oneh", [128, 1], BF, kind="ExternalInput")
    d_h0 = nc.dram_tensor("h0", [128, HC], F32, kind="ExternalInput")
    d_h0s = nc.dram_tensor("h0s", [128, CS], BF, kind="ExternalInput")
    d_enc = nc.dram_tensor("enc", [L, H], BF, kind="ExternalInput")
    d_attnw = nc.dram_tensor("attnw", [128, 32 * L], BF, kind="ExternalInput")
    d_attnb = nc.dram_tensor("attnb", [L, 1], F32, kind="ExternalInput")
    d_comb = nc.dram_tensor("comb", [128, 32 * CS * 128], BF, kind="ExternalInput")
    d_combb = nc.dram_tensor("combb", [128, CS], F32, kind="ExternalInput")
    d_ih = nc.dram_tensor("ihw", [128, CS * 48 * 128], BF, kind="ExternalInput")
    d_hh = nc.dram_tensor("hhw", [128, CS * 48 * 128], BF, kind="ExternalInput")
    d_gbias = nc.dram_tensor("gbias", [128, 64], F32, kind="ExternalInput")
    d_ow0 = nc.dram_tensor("outw0", [8, 2, 128, C0], BF, kind="ExternalInput")
    d_ow1 = nc.dram_tensor("outw1", [8, 2, 128, C1], BF, kind="ExternalInput")
    d_outb = nc.dram_tensor("outb", [1, VS], BF, kind="ExternalInput")

    d_lp = nc.dram_tensor("lp", [1, VS], F32, kind="ExternalOutput")
    d_h = nc.dram_tensor("h", [128, HC], F32, kind="ExternalOutput")
    d_aw = nc.dram_tensor("aw", [L, 1], F32, kind="ExternalOutput")

    with tile.TileContext(nc) as tc:
        with tc.tile_pool(name="wts", bufs=1) as wts, \
             tc.tile_pool(name="vec", bufs=1) as vec, \
             tc.tile_pool(name="ring", bufs=4) as ring, \
             tc.tile_pool(name="ps_sm", bufs=1, space="PSUM") as ps_sm, \
             tc.tile_pool(name="ps_j", bufs=1, space="PSUM") as ps_j, \
             tc.tile_pool(name="dram", bufs=1, space="DRAM") as dram:

            # ---- dummy collective first: absorbs start skew + cold mesh --
            cc_din = dram.tile([1, 1], F32)
            cc_dout = dram.tile([1, 1], F32)
            nc.gpsimd.collective_compute(
                "AllReduce", mybir.AluOpType.add,
                replica_groups=[list(range(N_CORES))],
                ins=[cc_din.opt()], outs=[cc_dout.opt()])

            # ---- small-path weight DMAs (gpsimd SWDGE, program order) ----
            sb_page = wts.tile([128, H], BF)
            nc.gpsimd.dma_start(sb_page[:], d_page[:])
            sb_oneh = wts.tile([128, 1], BF)
            nc.gpsimd.dma_start(sb_oneh[:], d_oneh[:])
            sb_h0 = wts.tile([128, HC], F32)
            nc.gpsimd.dma_start(sb_h0[:], d_h0[:])
            sb_h0s = wts.tile([128, CS], BF)
            nc.gpsimd.dma_start(sb_h0s[:], d_h0s[:])
            sb_attnw = wts.tile([128, 32 * L], BF)
            nc.gpsimd.dma_start(sb_attnw[:], d_attnw[:])
            sb_attnb = wts.tile([L, 1], F32)
            nc.gpsimd.dma_start(sb_attnb[:], d_attnb[:])
            sb_enc = wts.tile([L, H], BF)
            nc.gpsimd.dma_start(sb_enc[:], d_enc[:])
            sb_comb = wts.tile([128, 32 * CS * 128], BF)
            nc.gpsimd.dma_start(sb_comb[:], d_comb[:])
            sb_combb = wts.tile([128, CS], F32)
            nc.gpsimd.dma_start(sb_combb[:], d_combb[:])
            sb_ih = wts.tile([128, CS * 48 * 128], BF)
            nc.gpsimd.dma_start(sb_ih[:], d_ih[:])
            sb_hh = wts.tile([128, CS * 48 * 128], BF)
            nc.gpsimd.dma_start(sb_hh[:], d_hh[:])
            sb_gbias = wts.tile([128, 64], F32)
            nc.gpsimd.dma_start(sb_gbias[:], d_gbias[:])
            sb_outb = wts.tile([1, VS], BF)
            nc.gpsimd.dma_start(sb_outb[:], d_outb[:])

            # constants
            ones_c = wts.tile([128, 1], F32)
            nc.vector.memset(ones_c[:], 1.0)
            ones_r = wts.tile([1, 128], F32)
            nc.vector.memset(ones_r[:], 1.0)
            one1b = wts.tile([1, 1], BF)
            nc.vector.memset(one1b[:], 1.0)

            # ---- out_W ring DMAs on sync engine (own HWDGE ring) --------
            ring_tiles = {}
            for p, (d_ow, Cp) in enumerate([(d_ow0, C0), (d_ow1, C1)]):
                for g in range(8):
                    rt = ring.tile([128, 2 * C0], BF, tag="ring")
                    src = d_ow[g].rearrange("two prt c -> prt two c")
                    dst = rt[:, :2 * Cp].rearrange("prt (two c) -> prt two c",
                                                   two=2)
                    nc.sync.dma_start(dst, src)
                    ring_tiles[(p, g)] = rt

            # ---- embedding select: emb row via one-hot ------------------
            ps_emb = ps_sm.tile([128, HC], F32, tag="sm")
            for c in range(HC):
                nc.tensor.matmul(ps_emb[:, c:c + 1],
                                 sb_page[:, c * 128:(c + 1) * 128],
                                 sb_oneh[:], start=True, stop=True)
            emb_b = vec.tile([128, HC], BF)
            nc.vector.tensor_copy(emb_b[:], ps_emb[:])
            h0_b = vec.tile([128, HC], BF)
            nc.vector.tensor_copy(h0_b[:], sb_h0[:])

            # ---- attention logits [100,1] (W stationary) ----------------
            ps_al = ps_sm.tile([L, 1], F32, tag="sm")
            for c in range(32):
                rhs = emb_b[:, c:c + 1] if c < HC else h0_b[:, c - HC:c - HC + 1]
                nc.tensor.matmul(ps_al[:],
                                 sb_attnw[:, c * L:(c + 1) * L],
                                 rhs, start=(c == 0), stop=(c == 31))
            # softmax over 100 partitions (no max-shift; logits are O(1))
            al_e = vec.tile([L, 1], F32)
            nc.scalar.activation(al_e[:], ps_al[:], AF.Exp, bias=sb_attnb[:])
            ps_s = ps_sm.tile([1, 1], F32, tag="sm")
            nc.tensor.matmul(ps_s[:], al_e[:], ones_c[:L, :], start=True, stop=True)
            rs = vec.tile([1, 1], F32)
            nc.vector.reciprocal(rs[:], ps_s[:])
            ps_rb = ps_sm.tile([L, 1], F32, tag="sm")
            nc.tensor.matmul(ps_rb[:], ones_r[:, :L], rs[:], start=True, stop=True)
            aw_f = vec.tile([L, 1], F32)
            nc.vector.tensor_mul(aw_f[:], al_e[:], ps_rb[:])
            nc.gpsimd.dma_start(d_aw[:], aw_f[:])
            aw_b = vec.tile([L, 1], BF)
            nc.vector.tensor_copy(aw_b[:], aw_f[:])

            # ---- context = attn_weights @ enc, partition-major ----------
            ps_ctx = ps_sm.tile([128, HC], F32, tag="sm")
            for c in range(HC):
                nc.tensor.matmul(ps_ctx[:, c:c + 1],
                                 sb_enc[:, c * 128:(c + 1) * 128],
                                 aw_b[:], start=True, stop=True)
            ctx_b = vec.tile([128, HC], BF)
            nc.vector.tensor_copy(ctx_b[:], ps_ctx[:])

            # ---- rnn_input slice (comb output-sharded) ------------------
            ps_rnn = ps_sm.tile([128, CS], F32, tag="sm")
            for m in range(CS):
                for k in range(32):
                    rhs = emb_b[:, k:k + 1] if k < HC else ctx_b[:, k - HC:k - HC + 1]
                    nc.tensor.matmul(ps_rnn[:, m:m + 1],
                                     sb_comb[:, (k * CS + m) * 128:(k * CS + m + 1) * 128],
                                     rhs, start=(k == 0), stop=(k == 31))
            rnn_f = vec.tile([128, CS], F32)
            nc.vector.tensor_add(rnn_f[:], ps_rnn[:], sb_combb[:])
            rnn_b = vec.tile([128, CS], BF)
            nc.vector.tensor_copy(rnn_b[:], rnn_f[:])

            # ---- GRU partial gates (K-sharded) --------------------------
            ps_gi = ps_j.tile([128, 48], F32, tag="j0")
            ps_gh = ps_j.tile([128, 48], F32, tag="j1")
            for m in range(48):
                for k in range(CS):
                    nc.tensor.matmul(ps_gi[:, m:m + 1],
                                     sb_ih[:, (k * 48 + m) * 128:(k * 48 + m + 1) * 128],
                                     rnn_b[:, k:k + 1],
                                     start=(k == 0), stop=(k == CS - 1))
            for m in range(48):
                for k in range(CS):
                    nc.tensor.matmul(ps_gh[:, m:m + 1],
                                     sb_hh[:, (k * 48 + m) * 128:(k * 48 + m + 1) * 128],
                                     sb_h0s[:, k:k + 1],
                                     start=(k == 0), stop=(k == CS - 1))
            ghs = vec.tile([128, 48], F32)
            nc.scalar.copy(ghs[:], ps_gh[:])
            gpart = vec.tile([128, 64], F32)
            nc.vector.tensor_add(gpart[:, 0:16], ps_gi[:, 0:16], ghs[:, 0:16])
            nc.vector.tensor_add(gpart[:, 16:32], ps_gi[:, 16:32], ghs[:, 16:32])
            nc.vector.tensor_copy(gpart[:, 32:48], ps_gi[:, 32:48])
            nc.vector.tensor_copy(gpart[:, 48:64], ghs[:, 32:48])

            # ---- AllReduce the partial gates (bounces on scalar HWDGE) --
            cc_gin = dram.tile([128, 64], F32)
            cc_gout = dram.tile([128, 64], F32)
            nc.scalar.dma_start(cc_gin[:], gpart[:])
            nc.gpsimd.collective_compute(
                "AllReduce", mybir.AluOpType.add,
                replica_groups=[list(range(N_CORES))],
                ins=[cc_gin.opt()], outs=[cc_gout.opt()])
            gfull = vec.tile([128, 64], F32)
            nc.scalar.dma_start(gfull[:], cc_gout[:])
            gb = vec.tile([128, 64], F32)
            nc.vector.tensor_add(gb[:], gfull[:], sb_gbias[:])

            # ---- gates + new hidden state -------------------------------
            r_t = vec.tile([128, 16], F32)
            nc.scalar.activation(r_t[:], gb[:, 0:16], AF.Sigmoid)
            z_t = vec.tile([128, 16], F32)
            nc.scalar.activation(z_t[:], gb[:, 16:32], AF.Sigmoid)
            rd = vec.tile([128, 16], F32)
            nc.vector.tensor_mul(rd[:], r_t[:], gb[:, 48:64])
            cn = vec.tile([128, 16], F32)
            nc.vector.tensor_add(cn[:], gb[:, 32:48], rd[:])
            n_t = vec.tile([128, 16], F32)
            nc.scalar.activation(n_t[:], cn[:], AF.Tanh)
            hmn = vec.tile([128, 16], F32)
            nc.vector.tensor_sub(hmn[:], sb_h0[:], n_t[:])
            zh = vec.tile([128, 16], F32)
            nc.vector.tensor_mul(zh[:], z_t[:], hmn[:])
            h_f = vec.tile([128, 16], F32)
            nc.vector.tensor_add(h_f[:], n_t[:], zh[:])
            nc.gpsimd.dma_start(d_h[:], h_f[:])
            h_b = vec.tile([128, 16], BF)
            nc.vector.tensor_copy(h_b[:], h_f[:])

            # ---- vocab matmul: h stationary, W moving (N<=512) ----------
            # Logits are O(1) for this model, so log-softmax runs without
            # the max shift: exp+accumulate per tile, overlapped under PE.
            ls = vec.tile([1, VS], F32)
            esc = vec.tile([1, 512], F32)
            sacc = vec.tile([1, 13], F32)
            for p, (tiles_p, Cp) in enumerate([(PASS0, C0), (PASS1, C1)]):
                base = tiles_p[0][0] * 512
                psts = [ps_j.tile([1, w], F32, tag=f"j{i}", name=f"ps_p{p}j{i}")
                        for i, (j, w) in enumerate(tiles_p)]
                for g in range(8):
                    rt = ring_tiles[(p, g)]
                    for kk in range(2):
                        k = 2 * g + kk
                        for i, (j, w) in enumerate(tiles_p):
                            off = kk * Cp + (j * 512 - base)
                            nc.tensor.matmul(psts[i][:],
                                             h_b[:, k:k + 1],
                                             rt[:, off:off + w],
                                             start=(k == 0), stop=False)
                for i, (j, w) in enumerate(tiles_p):
                    # fold out_b in as a K=1 accumulation row
                    nc.tensor.matmul(psts[i][:], one1b[:],
                                     sb_outb[:, j * 512:j * 512 + w],
                                     start=False, stop=True)
                    nc.scalar.copy(ls[:, j * 512:j * 512 + w], psts[i][:])
                    nc.scalar.activation(esc[:, :w], psts[i][:], AF.Exp,
                                         accum_out=sacc[:, j:j + 1])

            s_i = vec.tile([1, 1], F32)
            nc.vector.reduce_sum(s_i[:], sacc[:], axis=AX)

            # ---- AllGather exp-sums, combine globally -------------------
            cc_sin = dram.tile([1, 1], F32)
            cc_sout = dram.tile([N_CORES, 1], F32)
            nc.scalar.dma_start(cc_sin[:], s_i[:])
            nc.gpsimd.collective_compute(
                "AllGather", mybir.AluOpType.bypass,
                replica_groups=[list(range(N_CORES))],
                ins=[cc_sin.opt()], outs=[cc_sout.opt()])
            st = vec.tile([1, N_CORES], F32)
            nc.scalar.dma_start(st[:], cc_sout[:])
            gsum = vec.tile([1, 1], F32)
            nc.vector.reduce_sum(gsum[:], st[:], axis=AX)
            logz = vec.tile([1, 1], F32)
            nc.scalar.activation(logz[:], gsum[:], AF.Ln)
            nlogz = vec.tile([1, 1], F32)
            nc.scalar.mul(nlogz[:], logz[:], -1.0)
            # logp = ls - logZ, split across DVE and ACT halves, in place
            HLF = 3072
            nc.vector.tensor_scalar_sub(ls[:, :HLF], ls[:, :HLF], logz[:])
            nc.scalar.activation(ls[:, HLF:], ls[:, HLF:], AF.Identity,
                                 bias=nlogz[:])
            nc.gpsimd.dma_start(d_lp[:], ls[:])

    nc.compile()
    return nc


def _pm(x):
    """[n*128] -> [128, n] partition-major."""
    n = x.shape[-1] // 128
    return np.ascontiguousarray(x.reshape(n, 128).T)


def _prep(input_tok, hidden, encoder_outputs, emb, attn_W, attn_b,
          comb_W, comb_b, W_ih, b_ih, W_hh, b_hh, out_W, out_b):
    tok = int(np.asarray(input_tok).reshape(-1)[0])
    blk = min((tok // 128) * 128, V - 128)
    page = np.ascontiguousarray(emb[blk:blk + 128]).astype(BF16)
    oneh = np.zeros((128, 1), BF16)
    oneh[tok - blk, 0] = 1.0

    h0 = np.asarray(hidden, np.float32).reshape(H)
    h0_pm = _pm(h0)

    attnw_t = np.ascontiguousarray(
        np.asarray(attn_W, np.float32).reshape(32, 128, L)
        .transpose(1, 0, 2).reshape(128, 32 * L)).astype(BF16)
    attnb_t = np.asarray(attn_b, np.float32).reshape(L, 1)
    enc = np.ascontiguousarray(np.asarray(encoder_outputs, np.float32)).astype(BF16)

    # padded out_W/out_b
    oW = np.zeros((H, VP), np.float32)
    oW[:, :V] = out_W
    ob = np.full((VP,), NEG, np.float32)
    ob[:V] = out_b

    gb = np.zeros((64 * 128,), np.float32)
    gb[0:2048] = b_ih[0:H] + b_hh[0:H]
    gb[2048:4096] = b_ih[H:2 * H] + b_hh[H:2 * H]
    gb[4096:6144] = b_ih[2 * H:3 * H]
    gb[6144:8192] = b_hh[2 * H:3 * H]
    gbias = _pm(gb)

    in_maps = []
    for i in range(N_CORES):
        ci = slice(i * 256, (i + 1) * 256)
        comb_i = np.ascontiguousarray(
            np.asarray(comb_W[:, ci], np.float32).reshape(32, 128, CS, 128)
            .transpose(1, 0, 2, 3).reshape(128, 32 * CS * 128)).astype(BF16)
        combb_i = _pm(np.asarray(comb_b[ci], np.float32))
        ih_i = np.ascontiguousarray(
            np.asarray(W_ih[:, ci], np.float32).T.reshape(CS, 128, 48, 128)
            .transpose(1, 0, 2, 3).reshape(128, CS * 48 * 128)).astype(BF16)
        hh_i = np.ascontiguousarray(
            np.asarray(W_hh[:, ci], np.float32).T.reshape(CS, 128, 48, 128)
            .transpose(1, 0, 2, 3).reshape(128, CS * 48 * 128)).astype(BF16)
        oWi = oW[:, i * VS:(i + 1) * VS]
        ow0 = np.ascontiguousarray(
            oWi[:, :C0].reshape(8, 2, 128, C0)).astype(BF16)
        ow1 = np.ascontiguousarray(
            oWi[:, C0:].reshape(8, 2, 128, C1)).astype(BF16)
        ob_i = np.ascontiguousarray(ob[i * VS:(i + 1) * VS].reshape(1, VS)).astype(BF16)
        h0s_i = np.ascontiguousarray(h0_pm[:, 2 * i:2 * i + 2]).astype(BF16)
        in_maps.append({
            "page": page, "oneh": oneh, "h0": h0_pm, "h0s": h0s_i,
            "enc": enc, "attnw": attnw_t, "attnb": attnb_t,
            "comb": comb_i, "combb": combb_i, "ihw": ih_i, "hhw": hh_i,
            "gbias": gbias, "outw0": ow0, "outw1": ow1, "outb": ob_i,
        })
    return in_maps


def run_spmd(in_maps, trace=False):
    from concourse.bass_utils import run_bass_kernel_spmd
    if "nc" not in _CACHE:
        _CACHE["nc"] = _build()
    return run_bass_kernel_spmd(_CACHE["nc"], in_maps,
                                core_ids=list(range(N_CORES)), trace=trace)


def kernel(input_tok, hidden, encoder_outputs, emb, attn_W, attn_b,
           comb_W, comb_b, W_ih, b_ih, W_hh, b_hh, out_W, out_b):
    in_maps = _prep(input_tok, hidden, encoder_outputs, emb, attn_W, attn_b,
                    comb_W, comb_b, W_ih, b_ih, W_hh, b_hh, out_W, out_b)
    res = run_spmd(in_maps)
    outs = res.results
    lp = np.concatenate([outs[i]["lp"].reshape(-1) for i in range(N_CORES)])
    logp = lp[:V].reshape(1, V).astype(np.float32)
    h_new = outs[0]["h"].T.reshape(1, 1, H).astype(np.float32)
    attnw = outs[0]["aw"].reshape(1, L).astype(np.float32)
    return logp, h_new, attnw


# revision 9
# speedup vs baseline: 1.1464x; 1.0991x over previous
"""AttnDecoderRNN single-step kernel for 8 TRN2 NeuronCores.

Tensor-parallel sharding (vocab-TP per the hint):
  - out_W/out_b sharded along vocab (padded 50257 -> 51200, 6400/core),
    log_softmax via a global-stats AllGather (max & exp-sum).
  - comb_W output-sharded (256 H-cols/core); GRU W_ih/W_hh K-sharded on the
    matching 256-wide rnn/h slice, partial gates combined with one AllReduce.
  - Small attention path (attn_W, encoder_outputs) replicated; embedding row
    selected on-device from a 128-row page via a one-hot matmul.

Small-path vectors live partition-major ([128, n] tiles). The vocab matmul
keeps h stationary and streams out_W as the moving operand (N=512), with
out_b folded in as an extra K=1 accumulation row; logits land on the free
dim of partition 0 where the local softmax stats are computed. Weights are
host-pre-tiled into SBUF-ready layouts, big ones cast to bf16 (f32 PSUM).
"""

import numpy as np
import ml_dtypes

H = 2048
V = 50257
L = 100
N_CORES = 8
VP = 51200          # padded vocab
VS = VP // N_CORES  # 6400 per core
KC = H // 128       # 16 k-chunks
HC = H // 128       # 16 h chunks
CS = H // N_CORES // 128  # 2 chunks of 128 = per-core 256-slice
NEG = -1.0e30

# vocab j-tiles: 12x512 + 1x256, split into two PSUM passes (7 + 6 tiles)
TILES = [(j, 512) for j in range(12)] + [(12, 256)]
PASS0 = TILES[:7]           # cols [0, 3584)
PASS1 = TILES[7:]           # cols [3584, 6400)
C0 = sum(w for _, w in PASS0)   # 3584
C1 = sum(w for _, w in PASS1)   # 2816

BF16 = ml_dtypes.bfloat16
FP8 = ml_dtypes.float8_e4m3

_CACHE = {}


def _build():
    import concourse.bacc as bacc
    import concourse.tile as tile
    import concourse.mybir as mybir

    F32 = mybir.dt.float32
    BF = mybir.dt.bfloat16
    AX = mybir.AxisListType.X
    AF = mybir.ActivationFunctionType

    nc = bacc.Bacc("TRN2", target_bir_lowering=False, debug=False,
                   num_devices=N_CORES)

    # ---- I/O -----------------------------------------------------------
    d_page = nc.dram_tensor("page", [128, H], BF, kind="ExternalInput")
    d_oneh = nc.dram_tensor("oneh", [128, 1], BF, kind="ExternalInput")
    d_h0 = nc.dram_tensor("h0", [128, HC], F32, kind="ExternalInput")
    d_h0s = nc.dram_tensor("h0s", [128, CS], BF, kind="ExternalInput")
    d_enc = nc.dram_tensor("enc", [L, H], BF, kind="ExternalInput")
    d_attnw = nc.dram_tensor("attnw", [128, 32 * L], BF, kind="ExternalInput")
    d_attnb = nc.dram_tensor("attnb", [L, 1], F32, kind="ExternalInput")
    d_comb = nc.dram_tensor("comb", [128, 32 * CS * 128], BF, kind="ExternalInput")
    d_combb = nc.dram_tensor("combb", [128, CS], F32, kind="ExternalInput")
    d_ih = nc.dram_tensor("ihw", [128, CS * 48 * 128], BF, kind="ExternalInput")
    d_hh = nc.dram_tensor("hhw", [128, CS * 48 * 128], BF, kind="ExternalInput")
    d_gbias = nc.dram_tensor("gbias", [128, 64], F32, kind="ExternalInput")
    F8 = mybir.dt.float8e4
    d_ow0 = nc.dram_tensor("outw0", [8, 2, 128, C0], F8, kind="ExternalInput")
    d_ow1 = nc.dram_tensor("outw1", [8, 2, 128, C1], F8, kind="ExternalInput")
    d_outb = nc.dram_tensor("outb", [1, VS], BF, kind="ExternalInput")

    d_lp = nc.dram_tensor("lp", [1, VS], F32, kind="ExternalOutput")
    d_h = nc.dram_tensor("h", [128, HC], F32, kind="ExternalOutput")
    d_aw = nc.dram_tensor("aw", [L, 1], F32, kind="ExternalOutput")

    with tile.TileContext(nc) as tc:
        with tc.tile_pool(name="wts", bufs=1) as wts, \
             tc.tile_pool(name="vec", bufs=1) as vec, \
             tc.tile_pool(name="ring", bufs=8) as ring, \
             tc.tile_pool(name="ps_sm", bufs=1, space="PSUM") as ps_sm, \
             tc.tile_pool(name="ps_j", bufs=1, space="PSUM") as ps_j, \
             tc.tile_pool(name="dram", bufs=1, space="DRAM") as dram:

            # ---- dummy collective first: absorbs start skew + cold mesh --
            cc_din = dram.tile([1, 1], F32)
            cc_dout = dram.tile([1, 1], F32)
            nc.gpsimd.collective_compute(
                "AllReduce", mybir.AluOpType.add,
                replica_groups=[list(range(N_CORES))],
                ins=[cc_din.opt()], outs=[cc_dout.opt()])

            # ---- small-path weight DMAs (gpsimd SWDGE, program order) ----
            sb_page = wts.tile([128, H], BF)
            nc.gpsimd.dma_start(sb_page[:], d_page[:])
            sb_oneh = wts.tile([128, 1], BF)
            nc.gpsimd.dma_start(sb_oneh[:], d_oneh[:])
            sb_h0 = wts.tile([128, HC], F32)
            nc.gpsimd.dma_start(sb_h0[:], d_h0[:])
            sb_h0s = wts.tile([128, CS], BF)
            nc.gpsimd.dma_start(sb_h0s[:], d_h0s[:])
            sb_attnw = wts.tile([128, 32 * L], BF)
            nc.gpsimd.dma_start(sb_attnw[:], d_attnw[:])
            sb_attnb = wts.tile([L, 1], F32)
            nc.gpsimd.dma_start(sb_attnb[:], d_attnb[:])
            sb_enc = wts.tile([L, H], BF)
            nc.gpsimd.dma_start(sb_enc[:], d_enc[:])
            sb_comb = wts.tile([128, 32 * CS * 128], BF)
            nc.gpsimd.dma_start(sb_comb[:], d_comb[:])
            sb_combb = wts.tile([128, CS], F32)
            nc.gpsimd.dma_start(sb_combb[:], d_combb[:])
            sb_ih = wts.tile([128, CS * 48 * 128], BF)
            nc.gpsimd.dma_start(sb_ih[:], d_ih[:])
            sb_hh = wts.tile([128, CS * 48 * 128], BF)
            nc.gpsimd.dma_start(sb_hh[:], d_hh[:])
            sb_gbias = wts.tile([128, 64], F32)
            nc.gpsimd.dma_start(sb_gbias[:], d_gbias[:])
            sb_outb = wts.tile([1, VS], BF)
            nc.gpsimd.dma_start(sb_outb[:], d_outb[:])

            # constants
            ones_c = wts.tile([128, 1], F32)
            nc.vector.memset(ones_c[:], 1.0)
            ones_r = wts.tile([1, 128], F32)
            nc.vector.memset(ones_r[:], 1.0)
            one1b = wts.tile([1, 1], BF)
            nc.vector.memset(one1b[:], 1.0)
            lnw = wts.tile([1, 1], F32)
            nc.scalar.activation(lnw[:], ones_c[:1, :], AF.Ln)

            # ---- out_W ring DMAs on sync engine (own HWDGE ring) --------
            ring_tiles = {}
            for p, (d_ow, Cp) in enumerate([(d_ow0, C0), (d_ow1, C1)]):
                for g in range(8):
                    rt = ring.tile([128, 2 * C0], F8, tag="ring")
                    src = d_ow[g].rearrange("two prt c -> prt two c")
                    dst = rt[:, :2 * Cp].rearrange("prt (two c) -> prt two c",
                                                   two=2)
                    nc.sync.dma_start(dst, src)
                    ring_tiles[(p, g)] = rt

            # ---- embedding select: emb row via one-hot ------------------
            ps_emb = ps_sm.tile([128, HC], F32, tag="sm")
            for c in range(HC):
                nc.tensor.matmul(ps_emb[:, c:c + 1],
                                 sb_page[:, c * 128:(c + 1) * 128],
                                 sb_oneh[:], start=True, stop=True)
            emb_b = vec.tile([128, HC], BF)
            nc.vector.tensor_copy(emb_b[:], ps_emb[:])
            h0_b = vec.tile([128, HC], BF)
            nc.vector.tensor_copy(h0_b[:], sb_h0[:])

            # ---- attention logits [100,1] (W stationary) ----------------
            ps_al = ps_sm.tile([L, 1], F32, tag="sm")
            for c in range(32):
                rhs = emb_b[:, c:c + 1] if c < HC else h0_b[:, c - HC:c - HC + 1]
                nc.tensor.matmul(ps_al[:],
                                 sb_attnw[:, c * L:(c + 1) * L],
                                 rhs, start=(c == 0), stop=(c == 31))
            # softmax over 100 partitions (no max-shift; logits are O(1))
            al_e = vec.tile([L, 1], F32)
            nc.scalar.activation(al_e[:], ps_al[:], AF.Exp, bias=sb_attnb[:])
            ps_s = ps_sm.tile([1, 1], F32, tag="sm")
            nc.tensor.matmul(ps_s[:], al_e[:], ones_c[:L, :], start=True, stop=True)
            rs = vec.tile([1, 1], F32)
            nc.vector.reciprocal(rs[:], ps_s[:])
            ps_rb = ps_sm.tile([L, 1], F32, tag="sm")
            nc.tensor.matmul(ps_rb[:], ones_r[:, :L], rs[:], start=True, stop=True)
            aw_f = vec.tile([L, 1], F32)
            nc.vector.tensor_mul(aw_f[:], al_e[:], ps_rb[:])
            nc.gpsimd.dma_start(d_aw[:], aw_f[:])
            aw_b = vec.tile([L, 1], BF)
            nc.vector.tensor_copy(aw_b[:], aw_f[:])

            # ---- context = attn_weights @ enc, partition-major ----------
            ps_ctx = ps_sm.tile([128, HC], F32, tag="sm")
            for c in range(HC):
                nc.tensor.matmul(ps_ctx[:, c:c + 1],
                                 sb_enc[:, c * 128:(c + 1) * 128],
                                 aw_b[:], start=True, stop=True)
            ctx_b = vec.tile([128, HC], BF)
            nc.vector.tensor_copy(ctx_b[:], ps_ctx[:])

            # ---- rnn_input slice (comb output-sharded) ------------------
            ps_rnn = ps_sm.tile([128, CS], F32, tag="sm")
            for m in range(CS):
                for k in range(32):
                    rhs = emb_b[:, k:k + 1] if k < HC else ctx_b[:, k - HC:k - HC + 1]
                    nc.tensor.matmul(ps_rnn[:, m:m + 1],
                                     sb_comb[:, (k * CS + m) * 128:(k * CS + m + 1) * 128],
                                     rhs, start=(k == 0), stop=(k == 31))
            rnn_f = vec.tile([128, CS], F32)
            nc.vector.tensor_add(rnn_f[:], ps_rnn[:], sb_combb[:])
            rnn_b = vec.tile([128, CS], BF)
            nc.vector.tensor_copy(rnn_b[:], rnn_f[:])

            # ---- GRU partial gates (K-sharded) --------------------------
            ps_gi = ps_j.tile([128, 48], F32, tag="j0")
            ps_gh = ps_j.tile([128, 48], F32, tag="j1")
            for m in range(48):
                for k in range(CS):
                    nc.tensor.matmul(ps_gi[:, m:m + 1],
                                     sb_ih[:, (k * 48 + m) * 128:(k * 48 + m + 1) * 128],
                                     rnn_b[:, k:k + 1],
                                     start=(k == 0), stop=(k == CS - 1))
            for m in range(48):
                for k in range(CS):
                    nc.tensor.matmul(ps_gh[:, m:m + 1],
                                     sb_hh[:, (k * 48 + m) * 128:(k * 48 + m + 1) * 128],
                                     sb_h0s[:, k:k + 1],
                                     start=(k == 0), stop=(k == CS - 1))
            ghs = vec.tile([128, 48], F32)
            nc.scalar.copy(ghs[:], ps_gh[:])
            gpart = vec.tile([128, 64], F32)
            nc.vector.tensor_add(gpart[:, 0:16], ps_gi[:, 0:16], ghs[:, 0:16])
            nc.vector.tensor_add(gpart[:, 16:32], ps_gi[:, 16:32], ghs[:, 16:32])
            nc.vector.tensor_copy(gpart[:, 32:48], ps_gi[:, 32:48])
            nc.vector.tensor_copy(gpart[:, 48:64], ghs[:, 32:48])

            # ---- AllReduce the partial gates (bounces on scalar HWDGE) --
            cc_gin = dram.tile([128, 64], F32)
            cc_gout = dram.tile([128, 64], F32)
            nc.scalar.dma_start(cc_gin[:], gpart[:])
            nc.gpsimd.collective_compute(
                "AllReduce", mybir.AluOpType.add,
                replica_groups=[list(range(N_CORES))],
                ins=[cc_gin.opt()], outs=[cc_gout.opt()])
            gfull = vec.tile([128, 64], F32)
            nc.scalar.dma_start(gfull[:], cc_gout[:])
            gb = vec.tile([128, 64], F32)
            nc.vector.tensor_add(gb[:], gfull[:], sb_gbias[:])

            # ---- gates + new hidden state -------------------------------
            r_t = vec.tile([128, 16], F32)
            nc.scalar.activation(r_t[:], gb[:, 0:16], AF.Sigmoid)
            z_t = vec.tile([128, 16], F32)
            nc.scalar.activation(z_t[:], gb[:, 16:32], AF.Sigmoid)
            rd = vec.tile([128, 16], F32)
            nc.vector.tensor_mul(rd[:], r_t[:], gb[:, 48:64])
            cn = vec.tile([128, 16], F32)
            nc.vector.tensor_add(cn[:], gb[:, 32:48], rd[:])
            n_t = vec.tile([128, 16], F32)
            nc.scalar.activation(n_t[:], cn[:], AF.Tanh)
            hmn = vec.tile([128, 16], F32)
            nc.vector.tensor_sub(hmn[:], sb_h0[:], n_t[:])
            zh = vec.tile([128, 16], F32)
            nc.vector.tensor_mul(zh[:], z_t[:], hmn[:])
            h_f = vec.tile([128, 16], F32)
            nc.vector.tensor_add(h_f[:], n_t[:], zh[:])
            nc.gpsimd.dma_start(d_h[:], h_f[:])
            h_b = vec.tile([128, 16], BF)
            nc.vector.tensor_copy(h_b[:], h_f[:])

            # ---- vocab matmul: h stationary, W moving (N<=512) ----------
            # Logits are O(1) for this model, so log-softmax runs without
            # the max shift: exp+accumulate per tile, overlapped under PE.
            ls = vec.tile([1, VS], F32)
            esc = vec.tile([1, 512], F32)
            sacc = vec.tile([1, 13], F32)
            for p, (tiles_p, Cp) in enumerate([(PASS0, C0), (PASS1, C1)]):
                base = tiles_p[0][0] * 512
                psts = [ps_j.tile([1, w], F32, tag=f"j{i}", name=f"ps_p{p}j{i}")
                        for i, (j, w) in enumerate(tiles_p)]
                for g in range(8):
                    rt = ring_tiles[(p, g)]
                    for kk in range(2):
                        k = 2 * g + kk
                        for i, (j, w) in enumerate(tiles_p):
                            off = kk * Cp + (j * 512 - base)
                            nc.tensor.matmul(psts[i][:],
                                             h_b[:, k:k + 1],
                                             rt[:, off:off + w],
                                             start=(k == 0), stop=False)
                for i, (j, w) in enumerate(tiles_p):
                    # fold out_b in as a K=1 accumulation row
                    nc.tensor.matmul(psts[i][:], one1b[:],
                                     sb_outb[:, j * 512:j * 512 + w],
                                     start=False, stop=True)
                    nc.vector.tensor_scalar_mul(ls[:, j * 512:j * 512 + w],
                                                psts[i][:], 1.0 / 64.0)
                    nc.scalar.activation(esc[:, :w], psts[i][:], AF.Exp,
                                         scale=1.0 / 64.0,
                                         accum_out=sacc[:, j:j + 1])

            s_i = vec.tile([1, 1], F32)
            nc.vector.reduce_sum(s_i[:], sacc[:], axis=AX)

            # ---- AllGather exp-sums, combine globally -------------------
            cc_sin = dram.tile([1, 1], F32)
            cc_sout = dram.tile([N_CORES, 1], F32)
            nc.scalar.dma_start(cc_sin[:], s_i[:])
            nc.gpsimd.collective_compute(
                "AllGather", mybir.AluOpType.bypass,
                replica_groups=[list(range(N_CORES))],
                ins=[cc_sin.opt()], outs=[cc_sout.opt()])
            st = vec.tile([1, N_CORES], F32)
            nc.scalar.dma_start(st[:], cc_sout[:])
            gsum = vec.tile([1, 1], F32)
            nc.vector.reduce_sum(gsum[:], st[:], axis=AX)
            logz = vec.tile([1, 1], F32)
            nc.scalar.activation(logz[:], gsum[:], AF.Ln)
            nlogz = vec.tile([1, 1], F32)
            nc.scalar.mul(nlogz[:], logz[:], -1.0)
            # logp = ls - logZ, split across DVE and ACT halves, in place
            HLF = 3904
            nc.vector.tensor_scalar_sub(ls[:, :HLF], ls[:, :HLF], logz[:])
            nc.scalar.activation(ls[:, HLF:], ls[:, HLF:], AF.Identity,
                                 bias=nlogz[:])
            nc.gpsimd.dma_start(d_lp[:], ls[:])

    nc.compile()
    return nc


def _pm(x):
    """[n*128] -> [128, n] partition-major."""
    n = x.shape[-1] // 128
    return np.ascontiguousarray(x.reshape(n, 128).T)


def _prep(input_tok, hidden, encoder_outputs, emb, attn_W, attn_b,
          comb_W, comb_b, W_ih, b_ih, W_hh, b_hh, out_W, out_b):
    tok = int(np.asarray(input_tok).reshape(-1)[0])
    blk = min((tok // 128) * 128, V - 128)
    page = np.ascontiguousarray(emb[blk:blk + 128]).astype(BF16)
    oneh = np.zeros((128, 1), BF16)
    oneh[tok - blk, 0] = 1.0

    h0 = np.asarray(hidden, np.float32).reshape(H)
    h0_pm = _pm(h0)

    attnw_t = np.ascontiguousarray(
        np.asarray(attn_W, np.float32).reshape(32, 128, L)
        .transpose(1, 0, 2).reshape(128, 32 * L)).astype(BF16)
    attnb_t = np.asarray(attn_b, np.float32).reshape(L, 1)
    enc = np.ascontiguousarray(np.asarray(encoder_outputs, np.float32)).astype(BF16)

    # padded out_W/out_b
    oW = np.zeros((H, VP), np.float32)
    oW[:, :V] = out_W
    ob = np.full((VP,), NEG, np.float32)
    ob[:V] = out_b

    gb = np.zeros((64 * 128,), np.float32)
    gb[0:2048] = b_ih[0:H] + b_hh[0:H]
    gb[2048:4096] = b_ih[H:2 * H] + b_hh[H:2 * H]
    gb[4096:6144] = b_ih[2 * H:3 * H]
    gb[6144:8192] = b_hh[2 * H:3 * H]
    gbias = _pm(gb)

    in_maps = []
    for i in range(N_CORES):
        ci = slice(i * 256, (i + 1) * 256)
        comb_i = np.ascontiguousarray(
            np.asarray(comb_W[:, ci], np.float32).reshape(32, 128, CS, 128)
            .transpose(1, 0, 2, 3).reshape(128, 32 * CS * 128)).astype(BF16)
        combb_i = _pm(np.asarray(comb_b[ci], np.float32))
        ih_i = np.ascontiguousarray(
            np.asarray(W_ih[:, ci], np.float32).T.reshape(CS, 128, 48, 128)
            .transpose(1, 0, 2, 3).reshape(128, CS * 48 * 128)).astype(BF16)
        hh_i = np.ascontiguousarray(
            np.asarray(W_hh[:, ci], np.float32).T.reshape(CS, 128, 48, 128)
            .transpose(1, 0, 2, 3).reshape(128, CS * 48 * 128)).astype(BF16)
        oWi = oW[:, i * VS:(i + 1) * VS]
        ow0 = np.ascontiguousarray(
            oWi[:, :C0].reshape(8, 2, 128, C0) * 64.0).astype(FP8)
        ow1 = np.ascontiguousarray(
            oWi[:, C0:].reshape(8, 2, 128, C1) * 64.0).astype(FP8)
        ob_i = np.ascontiguousarray(
            ob[i * VS:(i + 1) * VS].reshape(1, VS) * 64.0).astype(BF16)
        h0s_i = np.ascontiguousarray(h0_pm[:, 2 * i:2 * i + 2]).astype(BF16)
        in_maps.append({
            "page": page, "oneh": oneh, "h0": h0_pm, "h0s": h0s_i,
            "enc": enc, "attnw": attnw_t, "attnb": attnb_t,
            "comb": comb_i, "combb": combb_i, "ihw": ih_i, "hhw": hh_i,
            "gbias": gbias, "outw0": ow0, "outw1": ow1, "outb": ob_i,
        })
    return in_maps


def run_spmd(in_maps, trace=False):
    from concourse.bass_utils import run_bass_kernel_spmd
    if "nc" not in _CACHE:
        _CACHE["nc"] = _build()
    return run_bass_kernel_spmd(_CACHE["nc"], in_maps,
                                core_ids=list(range(N_CORES)), trace=trace)


def kernel(input_tok, hidden, encoder_outputs, emb, attn_W, attn_b,
           comb_W, comb_b, W_ih, b_ih, W_hh, b_hh, out_W, out_b):
    in_maps = _prep(input_tok, hidden, encoder_outputs, emb, attn_W, attn_b,
                    comb_W, comb_b, W_ih, b_ih, W_hh, b_hh, out_W, out_b)
    res = run_spmd(in_maps)
    outs = res.results
    lp = np.concatenate([outs[i]["lp"].reshape(-1) for i in range(N_CORES)])
    logp = lp[:V].reshape(1, V).astype(np.float32)
    h_new = outs[0]["h"].T.reshape(1, 1, H).astype(np.float32)
    attnw = outs[0]["aw"].reshape(1, L).astype(np.float32)
    return logp, h_new, attnw
